# revision 1
# baseline (speedup 1.0000x reference)
"""Trainium2 Bass kernel for gradual-int8 Conv2d (exact int8 GEMM blended with a
256x256 LUT GEMM).

Strategy
--------
Both branches of the reference are sums of a 256x256 table over the im2col
contraction:

    acc[n, o] = sum_j T_eff[qx[n,j] + 128, qw[o,j] + 128]
    T_eff     = (1 - ALPHA) * outer(q, q) + ALPHA * lut,   q = arange(256) - 128

(the exact int8 product a*b is itself a rank-1 table). Factorizing T_eff with an
SVD on the host (exact to fp32 precision; rank 1 for the reference's
product-table LUT) turns the LUT gathers into a handful of ordinary GEMMs:

    acc = sum_r (fx_r[qx + 128])-conv-(fw_r[qw + 128])

Each rank term is a 3x3 conv over mapped activations. Operands are split into
exact-bf16 hi/lo "parts" so the PE runs at bf16 rate with ~fp32 accuracy; when
the x-side table is a scaled integer table (true for the product-table LUT) it
is renormalized to exact integers and needs no lo term. Two parts are packed
per matmul on the 128-partition contraction dim (64 channels x 2 parts), so the
graded case runs 9 matmuls (one per conv tap) of [K=128, M=128, N=512]
accumulating in one PSUM bank, a fused bias-add/PSUM-evacuation on the vector
engine, and one output DMA.

Sharding: pure data parallel over the 4096 output pixels. Each of the 8 cores
computes a 16-row half of one batch image from a halo-padded input slice.
Weight-derived operands are replicated; there are no collectives.
"""

import numpy as np
import ml_dtypes

import concourse.bacc as bacc
import concourse.mybir as mybir
import concourse.tile as tile
from concourse.bass_utils import run_bass_kernel_spmd

# Problem constants (hardcoded per the harness contract).
B, C, H, W = 4, 64, 32, 32
O, KH, KW = 128, 3, 3
OH, OW = H, W          # stride 1, pad 1
QMAX = 127.0
ALPHA = 0.5
MOMENTUM = 0.05
N_CORES = 8
HH = OH // 2           # output rows per core (half an image)
NPIX = HH * OW         # 512 pixels per core = one PSUM bank
SROW = W + 2           # padded row length (34)
SLEN = (HH + 2) * SROW  # padded slice elems per channel (612)

BF16 = ml_dtypes.bfloat16

# Exposed for test harnesses: when True, run_bass_kernel_spmd is called with
# trace=True and the BassKernelResults lands in _LAST_RESULT.
_TRACE = False
_LAST_RESULT = None

_PROGRAM_CACHE = {}


def _factorize_table(lut: np.ndarray):
    """Factorize T_eff into rank-1 terms; returns (fx [256, r], fw [256, r])."""
    q = np.arange(256, dtype=np.float64) - 128.0
    T = (1.0 - ALPHA) * np.outer(q, q) + ALPHA * lut.astype(np.float64)
    U, S, Vt = np.linalg.svd(T)
    # Keep components above fp32-noise relative to the dominant one. Cap the
    # rank to keep the device program bounded for adversarial full-rank tables
    # (accuracy then degrades gracefully; the graded table is rank 1).
    if S[0] == 0.0:
        r = 1
    else:
        r = int(np.sum(S > S[0] * 1e-7))
        r = max(1, min(r, 64))
    s = np.sqrt(S[:r])
    fx = U[:, :r] * s
    fw = Vt[:r, :].T * s
    return fx, fw


def _nice_normalize(fx_r: np.ndarray, fw_r: np.ndarray):
    """If fx_r is a scaled integer table (|ints| <= 256), rescale so the x-side
    values are exact in bf16; fold the scale into the w-side."""
    a = np.abs(fx_r)
    nz = a[a > 1e-300]
    if nz.size == 0:
        return np.zeros_like(fx_r), fw_r
    beta = nz.min()
    scaled = fx_r / beta
    rounded = np.round(scaled)
    if np.max(np.abs(scaled - rounded)) < 1e-6 and np.max(np.abs(rounded)) <= 256:
        return rounded, fw_r * beta
    return fx_r, fw_r


def _bf16_terms(arr) -> list:
    """Split an array into 1-2 bf16 arrays summing to ~it."""
    a32 = np.asarray(arr, dtype=np.float32)
    hi = a32.astype(BF16)
    resid = a32 - hi.astype(np.float32)
    if not np.any(resid):
        return [hi]
    return [hi, resid.astype(BF16)]


def _build_program(G: int):
    """Single-core program: one packed input DMA, G*9 matmuls of
    [K=128, M=128, N=512] accumulating in one PSUM bank (each matmul contracts
    64 channels x 2 operand parts), fused bias-add on the PSUM read, store."""
    F = G * SLEN + G * KH * KW * O + 2
    OFF_W = G * SLEN
    OFF_B = OFF_W + G * KH * KW * O

    nc = bacc.Bacc("TRN2", target_bir_lowering=False, debug=False,
                   num_devices=N_CORES)
    xin_d = nc.dram_tensor("xin", [128, F], mybir.dt.bfloat16,
                           kind="ExternalInput")
    out_d = nc.dram_tensor("out", [O, NPIX], mybir.dt.float32,
                           kind="ExternalOutput")

    with tile.TileContext(nc) as tc:
        with tc.tile_pool(name="data", bufs=1) as pool, \
             tc.tile_pool(name="psum", bufs=2, space="PSUM") as psum_pool:
            xin_sb = pool.tile([128, F], mybir.dt.bfloat16)
            # Two byte-balanced DMAs on different engine queues (sync + the
            # scalar engine's) so the transfers overlap.
            split = F // 2
            nc.sync.dma_start(out=xin_sb[:, :split], in_=xin_d[:, :split])
            nc.scalar.dma_start(out=xin_sb[:, split:], in_=xin_d[:, split:])

            # The two trailing bf16 columns hold the f32 bias bit pattern.
            bias_ap = xin_sb[:, OFF_B:OFF_B + 2].bitcast(mybir.dt.float32)
            n_mm = G * KH * KW
            # Output rows in two uneven chunks: the first chunk's bias-add and
            # store overlap the second chunk's matmuls, and the exposed tail
            # after the last matmul is only the small chunk's.
            ROWS = (12, 4)
            r0 = 0
            for ch, nrows in enumerate(ROWS):
                npx = nrows * W
                ps = psum_pool.tile([O, npx], mybir.dt.float32,
                                    tag=f"ps{ch}")
                i = 0
                for g in range(G):
                    s_view = xin_sb[:, g * SLEN:(g + 1) * SLEN].rearrange(
                        "p (h w) -> p h w", h=HH + 2)
                    for t in range(KH * KW):
                        kh, kw = divmod(t, KW)
                        off = OFF_W + (g * KH * KW + t) * O
                        nc.tensor.matmul(
                            ps,
                            lhsT=xin_sb[:, off:off + O],
                            rhs=s_view[:, kh + r0:kh + r0 + nrows, kw:kw + W],
                            start=(i == 0), stop=(i == n_mm - 1))
                        i += 1
                o_sb = pool.tile([O, npx], mybir.dt.float32, tag=f"o_sb{ch}")
                nc.vector.tensor_scalar(
                    o_sb, ps, bias_ap, None, mybir.AluOpType.add)
                nc.sync.dma_start(
                    out=out_d[:, r0 * W:r0 * W + npx], in_=o_sb)
                r0 += nrows
    nc.compile()
    return nc


def _prepare(x, weight, bias, lut):
    """All host-side math: quantization, factorization, operand packing.
    Returns (G, in_maps)."""
    # --- Quantization (bit-exact replication of the f32 reference math) ---
    scale_x = np.float32(MOMENTUM) * (np.max(np.abs(x)) / np.float32(QMAX)) \
        + np.float32((1.0 - MOMENTUM) * 1.0)
    qx = np.clip(np.round(x / scale_x), -127.0, 127.0)            # [B,C,H,W]
    scale_w = np.max(np.abs(weight), axis=(1, 2, 3)) / np.float32(QMAX)  # [O]
    qw = np.clip(np.round(weight / scale_w[:, None, None, None]), -127.0, 127.0)

    # --- Table factorization ---
    fx, fw = _factorize_table(lut)
    rank = fx.shape[1]
    dequant = scale_x.astype(np.float64) * scale_w.astype(np.float64)  # [O]

    ix = qx.astype(np.int32) + 128
    ixpad = np.zeros((B, C, H + 2, W + 2), dtype=np.int32) + 128
    # Padded taps look up index 128 (qx = 0), NOT table value 0: the reference
    # pads qx with zeros before the LUT lookup, and lut[128, b] != 0 in general.
    ixpad[:, :, 1:-1, 1:-1] = ix
    iw = qw.astype(np.int32) + 128                                # [O,C,KH,KW]

    x_tables = []      # bf16 [256] lookup tables, one per x-side term
    parts = []         # (x_table_index, w_term bf16 [C, 9, O])
    for r in range(rank):
        fx_r, fw_r = _nice_normalize(fx[:, r], fw[:, r])
        # Folded w-side operand in f64: fw_r[qw+128] * scale_x * scale_w[o]
        lwf = fw_r[iw] * dequant[:, None, None, None]             # [O,C,KH,KW]
        lwf = lwf.transpose(1, 2, 3, 0).reshape(C, KH * KW, O)    # [C,9,O]
        w_terms = _bf16_terms(lwf)
        xt_terms = _bf16_terms(fx_r)
        both_split = len(w_terms) == 2 and len(xt_terms) == 2
        base = len(x_tables)
        x_tables.extend(xt_terms)
        for i_x in range(len(xt_terms)):
            for i_w, wt in enumerate(w_terms):
                if both_split and i_x == 1 and i_w == 1:
                    continue  # drop the lo*lo term (~2^-16 relative)
                parts.append((base + i_x, wt))

    NP = len(parts)
    G = (NP + 1) // 2
    F = G * SLEN + G * KH * KW * O + 2
    OFF_W = G * SLEN
    OFF_B = OFF_W + G * KH * KW * O

    bias_u16 = bias.astype("<f4").view("<u2").reshape(O, 2)

    # Mapped activations for every x-side term: [NX, B, C, H+2, W+2] bf16.
    xmaps = np.stack([t[ixpad] for t in x_tables], axis=0)

    # W chunks are identical across cores: [128, G*9*O] bf16.
    wreg = np.zeros((128, G * KH * KW * O), dtype=BF16)
    for p, (_, wt) in enumerate(parts):
        g, half = divmod(p, 2)
        rows = slice(half * C, half * C + C)
        for t in range(KH * KW):
            col = (g * KH * KW + t) * O
            wreg[rows, col:col + O] = wt[:, t, :]

    in_maps = []
    for c in range(N_CORES):
        b, half_img = divmod(c, 2)
        h0 = half_img * HH
        xin = np.zeros((128, F), dtype=BF16)
        for p, (xi, _) in enumerate(parts):
            g, half = divmod(p, 2)
            rows = slice(half * C, half * C + C)
            xin[rows, g * SLEN:(g + 1) * SLEN] = \
                xmaps[xi, b, :, h0:h0 + HH + 2, :].reshape(C, SLEN)
        xin[:, OFF_W:OFF_B] = wreg
        xin.view("<u2")[:O, OFF_B:OFF_B + 2] = bias_u16
        in_maps.append({"xin": xin})
    return G, in_maps


def kernel(x: np.ndarray, weight: np.ndarray, bias: np.ndarray,
           lut: np.ndarray) -> np.ndarray:
    global _LAST_RESULT
    x = np.asarray(x, dtype=np.float32)
    weight = np.asarray(weight, dtype=np.float32)
    bias = np.asarray(bias, dtype=np.float32)
    lut = np.asarray(lut, dtype=np.float32)

    G, in_maps = _prepare(x, weight, bias, lut)

    if G not in _PROGRAM_CACHE:
        _PROGRAM_CACHE[G] = _build_program(G)
    nc = _PROGRAM_CACHE[G]

    try:
        res = run_bass_kernel_spmd(nc, in_maps, core_ids=list(range(N_CORES)),
                                   trace=_TRACE)
    except ModuleNotFoundError:
        # NTFF profiling hooks absent in this environment; run untraced.
        res = run_bass_kernel_spmd(nc, in_maps, core_ids=list(range(N_CORES)),
                                   trace=False)
    _LAST_RESULT = res

    out = np.empty((B, O, OH, OW), dtype=np.float32)
    for c in range(N_CORES):
        b, half_img = divmod(c, 2)
        h0 = half_img * HH
        out[b, :, h0:h0 + HH, :] = res.results[c]["out"].reshape(O, HH, OW)
    return out



# revision 4
# speedup vs baseline: 1.2933x; 1.2933x over previous
"""Trainium2 Bass kernel for gradual-int8 Conv2d (exact int8 GEMM blended with a
256x256 LUT GEMM).

Fast path (the graded configuration)
------------------------------------
The reference's LUT is the exact product table lut[a+128, b+128] = a*b, so the
blended accumulator reduces to the plain quantized conv acc = qx (*) qw.  Two
further structural facts make this extremely cheap on the PE array:

* scale_x is momentum-damped toward 1.0, so qx = round(x / ~0.95) lies in
  [-8, 8] -- exactly representable in fp8e4m3 with no split.
* qw in [-127, 127] splits into balanced base-16 digits qw = 16*wh + wl with
  |wh|,|wl| <= 8; both digits (and 16*wh) are fp8e4m3-exact.

The conv then runs entirely in fp8 DoubleRow matmuls, which contract
2x128 = 256 operand pairs per output column at half a PE cycle per column:
partitions carry (channel x weight-digit), the DoubleRow pair dim carries two
conv taps at once via overlapping shifted windows of the padded image held
flat in SBUF (junk columns from the horizontal halo are computed and then
dropped during PSUM evacuation).  9 taps x 64 channels x 2 digits = 5
DoubleRow matmuls per pixel tile.  PSUM accumulates the exact integer conv
(< 2^24, so fp32-exact); a fused scale*psum+bias on the scalar engine
dequantizes straight out of PSUM into bf16 for the store.

Sharding: pure data parallel over the 4096 output pixels; each of 8 cores
computes a 16-row half of one batch image.  No collectives.

A generic slower path (SVD table factorization -> bf16 tap matmuls, from the
previous revision) handles any other LUT/scale regime.
"""

import numpy as np
import ml_dtypes

import concourse.bacc as bacc
import concourse.mybir as mybir
import concourse.tile as tile
from concourse.ap import AP
from concourse.bass_utils import run_bass_kernel_spmd

# Problem constants (hardcoded per the harness contract).
B, C, H, W = 4, 64, 32, 32
O, KH, KW = 128, 3, 3
OH, OW = H, W          # stride 1, pad 1
QMAX = 127.0
ALPHA = 0.5
MOMENTUM = 0.05
N_CORES = 8

BF16 = ml_dtypes.bfloat16
FP8 = ml_dtypes.float8_e4m3

# --- fast-path geometry ---
SROW = W + 2            # padded row length (34)
XROWS = 20              # padded image rows staged per core (16 out + halo + overread)
XLEN = XROWS * SROW     # 680 bytes per partition
ROWS_PER_CHUNK = 8      # two pixel chunks of 8 output rows each
NJ = ROWS_PER_CHUNK * SROW   # matmul moving columns per chunk (incl 2 junk cols/row)
# tap pairs per DoubleRow matmul: (flat window offset, j-dim stride)
# pair taps: {(0,0),(0,1)} {(1,0),(1,1)} {(2,0),(2,1)} {(0,2),(1,2)} {(2,2),-}
PAIRS = [(0, 1), (SROW, 1), (2 * SROW, 1), (2, SROW), (2 * SROW + 2, SROW)]
WCOLS = len(PAIRS) * 2 * O + 8   # 5 pairs x [j=2][m=128] fp8 + scale/bias f32

# Exposed for test harnesses: when True, run_bass_kernel_spmd is called with
# trace=True and the BassKernelResults lands in _LAST_RESULT.
_TRACE = False
_LAST_RESULT = None

_PROGRAM_CACHE = {}


def _build_fast_program():
    nc = bacc.Bacc("TRN2", target_bir_lowering=False, debug=False,
                   num_devices=N_CORES)
    x_d = nc.dram_tensor("xin", [128, XLEN], mybir.dt.float8e4,
                         kind="ExternalInput")
    w_d = nc.dram_tensor("win", [128, WCOLS], mybir.dt.float8e4,
                         kind="ExternalInput")
    out_d = nc.dram_tensor("out", [O, 2 * ROWS_PER_CHUNK * W], mybir.dt.bfloat16,
                           kind="ExternalOutput")

    with tile.TileContext(nc) as tc:
        with tc.tile_pool(name="data", bufs=1) as pool, \
             tc.tile_pool(name="psum", bufs=2, space="PSUM") as psum_pool:
            x_sb = pool.tile([128, XLEN], mybir.dt.float8e4)
            w_sb = pool.tile([128, WCOLS], mybir.dt.float8e4)
            # w on the SP HWDGE queue, x on the Pool SWDGE queue: the issue
            # latencies overlap and neither waits on the other.
            nc.sync.dma_start(out=w_sb[:, :], in_=w_d[:, :])
            nc.gpsimd.dma_start(out=x_sb[:, :], in_=x_d[:, :])

            sb32 = w_sb[:, WCOLS - 8:WCOLS].bitcast(mybir.dt.float32)
            scale_ap = sb32[:, 0:1]
            bias_ap = sb32[:, 1:2]

            x_flat = x_sb[:, :]          # AP over the whole x tile
            for ch in range(2):
                base = ch * ROWS_PER_CHUNK * SROW
                ps = psum_pool.tile([O, NJ], mybir.dt.float32, tag=f"ps{ch}")
                for p, (off, js) in enumerate(PAIRS):
                    rhs = AP(x_flat.tensor, x_flat.offset + base + off,
                             [list(x_flat.ap[0]), [js, 2], [1, NJ]])
                    lhsT = w_sb[:, p * 2 * O:(p + 1) * 2 * O].rearrange(
                        "p (j m) -> p j m", j=2)
                    nc.tensor.matmul(
                        ps, lhsT=lhsT, rhs=rhs,
                        start=(p == 0), stop=(p == len(PAIRS) - 1),
                        perf_mode=mybir.MatmulPerfMode.DoubleRow)
                # Fused dequant + bias on the scalar engine straight out of
                # PSUM, dropping the 2 junk columns per image row.
                o_sb = pool.tile([O, ROWS_PER_CHUNK * W], mybir.dt.bfloat16,
                                 tag=f"o{ch}")
                ps_v = ps.rearrange("p (h w) -> p h w", h=ROWS_PER_CHUNK)[:, :, 0:W]
                o_v = o_sb.rearrange("p (h w) -> p h w", h=ROWS_PER_CHUNK)
                nc.scalar.activation(
                    o_v, ps_v, mybir.ActivationFunctionType.Identity,
                    bias=bias_ap, scale=scale_ap)
                nc.sync.dma_start(
                    out=out_d[:, ch * ROWS_PER_CHUNK * W:(ch + 1) * ROWS_PER_CHUNK * W],
                    in_=o_sb)
    nc.compile()
    return nc


def _quantize(x, weight):
    """Bit-exact replication of the reference's f32 quantization math."""
    scale_x = np.float32(MOMENTUM) * (np.max(np.abs(x)) / np.float32(QMAX)) \
        + np.float32((1.0 - MOMENTUM) * 1.0)
    qx = np.clip(np.round(x / scale_x), -127.0, 127.0)            # [B,C,H,W]
    scale_w = np.max(np.abs(weight), axis=(1, 2, 3)) / np.float32(QMAX)  # [O]
    qw = np.clip(np.round(weight / scale_w[:, None, None, None]), -127.0, 127.0)
    return scale_x, qx, scale_w, qw


def _fast_path_ok(qx, lut):
    if np.max(np.abs(qx)) > 8:
        return False
    q = np.arange(256, dtype=np.float64) - 128.0
    # T_eff == outer(q, q) iff lut is the exact product table.
    return np.array_equal(np.asarray(lut, dtype=np.float64), np.outer(q, q))


def _prepare_fast(scale_x, qx, scale_w, qw, bias):
    # Padded quantized image in fp8, with 2 extra zero rows for the
    # benign overread of the half-filled tap pair.
    pad = np.zeros((B, C, H + 4, SROW), dtype=FP8)
    pad[:, :, 1:H + 1, 1:W + 1] = qx.astype(FP8)

    # Balanced base-16 digit split of qw: qw = 16*wh + wl, |wh|,|wl| <= 8.
    qwi = qw.astype(np.int32)
    wh = np.floor_divide(qwi + 8, 16)
    wl = qwi - 16 * wh
    # Operand values: digit 0 row-block carries 16*wh, digit 1 carries wl.
    wops = np.stack([(16 * wh), wl], axis=0).astype(FP8)  # [2, O, C, KH, KW]

    # w tensor: per pair p, [j (tap of pair), m] blocks over partitions c+64b.
    wbuf = np.zeros((128, WCOLS), dtype=FP8)
    pair_taps = [((0, 0), (0, 1)), ((1, 0), (1, 1)), ((2, 0), (2, 1)),
                 ((0, 2), (1, 2)), ((2, 2), None)]
    for p, taps in enumerate(pair_taps):
        for j, tap in enumerate(taps):
            if tap is None:
                continue
            kh, kw = tap
            col = p * 2 * O + j * O
            for b in range(2):
                # [O, C] -> rows c+64b, cols m
                wbuf[b * C:(b + 1) * C, col:col + O] = wops[b, :, :, kh, kw].T
    sb = np.empty((128, 2), dtype="<f4")
    sb[:, 0] = np.float32(scale_x) * scale_w.astype(np.float32)
    sb[:, 1] = bias.astype(np.float32)
    wbuf.view("<u1")[:, WCOLS - 8:WCOLS] = sb.view("<u1")

    in_maps = []
    for c in range(N_CORES):
        b_i, half = divmod(c, 2)
        h0 = half * (H // 2)
        xs = pad[b_i, :, h0:h0 + XROWS, :].reshape(C, XLEN)
        xin = np.concatenate([xs, xs], axis=0)  # both weight-digit halves
        in_maps.append({"xin": xin, "win": wbuf})
    return in_maps


def _run_fast(x, weight, bias, lut, scale_x, qx, scale_w, qw):
    global _LAST_RESULT
    in_maps = _prepare_fast(scale_x, qx, scale_w, qw, bias)
    if "fast" not in _PROGRAM_CACHE:
        _PROGRAM_CACHE["fast"] = _build_fast_program()
    nc = _PROGRAM_CACHE["fast"]
    try:
        res = run_bass_kernel_spmd(nc, in_maps, core_ids=list(range(N_CORES)),
                                   trace=_TRACE)
    except ModuleNotFoundError:
        res = run_bass_kernel_spmd(nc, in_maps, core_ids=list(range(N_CORES)),
                                   trace=False)
    _LAST_RESULT = res

    out = np.empty((B, O, OH, OW), dtype=np.float32)
    for c in range(N_CORES):
        b_i, half = divmod(c, 2)
        h0 = half * (H // 2)
        blk = res.results[c]["out"].astype(np.float32).reshape(O, H // 2, W)
        out[b_i, :, h0:h0 + H // 2, :] = blk
    return out


# ---------------------------------------------------------------------------
# Generic fallback (previous revision): SVD table factorization -> bf16 tap
# matmuls.  Used only if the LUT is not the exact product table or the
# activation quantization leaves the fp8-exact range.
# ---------------------------------------------------------------------------

HH = OH // 2           # output rows per core (half an image)
NPIX = HH * OW         # 512 pixels per core = one PSUM bank
GSLEN = (HH + 2) * SROW  # padded slice elems per channel (612)


def _factorize_table(lut: np.ndarray):
    """Factorize T_eff into rank-1 terms; returns (fx [256, r], fw [256, r])."""
    q = np.arange(256, dtype=np.float64) - 128.0
    T = (1.0 - ALPHA) * np.outer(q, q) + ALPHA * lut.astype(np.float64)
    U, S, Vt = np.linalg.svd(T)
    if S[0] == 0.0:
        r = 1
    else:
        r = int(np.sum(S > S[0] * 1e-7))
        r = max(1, min(r, 64))
    s = np.sqrt(S[:r])
    fx = U[:, :r] * s
    fw = Vt[:r, :].T * s
    return fx, fw


def _nice_normalize(fx_r: np.ndarray, fw_r: np.ndarray):
    a = np.abs(fx_r)
    nz = a[a > 1e-300]
    if nz.size == 0:
        return np.zeros_like(fx_r), fw_r
    beta = nz.min()
    scaled = fx_r / beta
    rounded = np.round(scaled)
    if np.max(np.abs(scaled - rounded)) < 1e-6 and np.max(np.abs(rounded)) <= 256:
        return rounded, fw_r * beta
    return fx_r, fw_r


def _bf16_terms(arr) -> list:
    a32 = np.asarray(arr, dtype=np.float32)
    hi = a32.astype(BF16)
    resid = a32 - hi.astype(np.float32)
    if not np.any(resid):
        return [hi]
    return [hi, resid.astype(BF16)]


def _build_generic_program(G: int):
    F = G * GSLEN + G * KH * KW * O + 2
    OFF_W = G * GSLEN
    OFF_B = OFF_W + G * KH * KW * O

    nc = bacc.Bacc("TRN2", target_bir_lowering=False, debug=False,
                   num_devices=N_CORES)
    xin_d = nc.dram_tensor("xin", [128, F], mybir.dt.bfloat16,
                           kind="ExternalInput")
    out_d = nc.dram_tensor("out", [O, NPIX], mybir.dt.float32,
                           kind="ExternalOutput")

    with tile.TileContext(nc) as tc:
        with tc.tile_pool(name="data", bufs=1) as pool, \
             tc.tile_pool(name="psum", bufs=2, space="PSUM") as psum_pool:
            xin_sb = pool.tile([128, F], mybir.dt.bfloat16)
            split = F // 2
            nc.sync.dma_start(out=xin_sb[:, :split], in_=xin_d[:, :split])
            nc.scalar.dma_start(out=xin_sb[:, split:], in_=xin_d[:, split:])

            bias_ap = xin_sb[:, OFF_B:OFF_B + 2].bitcast(mybir.dt.float32)
            n_mm = G * KH * KW
            ROWS = (12, 4)
            r0 = 0
            for ch, nrows in enumerate(ROWS):
                npx = nrows * W
                ps = psum_pool.tile([O, npx], mybir.dt.float32,
                                    tag=f"ps{ch}")
                i = 0
                for g in range(G):
                    s_view = xin_sb[:, g * GSLEN:(g + 1) * GSLEN].rearrange(
                        "p (h w) -> p h w", h=HH + 2)
                    for t in range(KH * KW):
                        kh, kw = divmod(t, KW)
                        off = OFF_W + (g * KH * KW + t) * O
                        nc.tensor.matmul(
                            ps,
                            lhsT=xin_sb[:, off:off + O],
                            rhs=s_view[:, kh + r0:kh + r0 + nrows, kw:kw + W],
                            start=(i == 0), stop=(i == n_mm - 1))
                        i += 1
                o_sb = pool.tile([O, npx], mybir.dt.float32, tag=f"o_sb{ch}")
                nc.vector.tensor_scalar(
                    o_sb, ps, bias_ap, None, mybir.AluOpType.add)
                nc.sync.dma_start(
                    out=out_d[:, r0 * W:r0 * W + npx], in_=o_sb)
                r0 += nrows
    nc.compile()
    return nc


def _prepare_generic(x, weight, bias, lut, scale_x, qx, scale_w, qw):
    fx, fw = _factorize_table(lut)
    rank = fx.shape[1]
    dequant = scale_x.astype(np.float64) * scale_w.astype(np.float64)  # [O]

    ix = qx.astype(np.int32) + 128
    ixpad = np.zeros((B, C, H + 2, W + 2), dtype=np.int32) + 128
    ixpad[:, :, 1:-1, 1:-1] = ix
    iw = qw.astype(np.int32) + 128                                # [O,C,KH,KW]

    x_tables = []
    parts = []
    for r in range(rank):
        fx_r, fw_r = _nice_normalize(fx[:, r], fw[:, r])
        lwf = fw_r[iw] * dequant[:, None, None, None]             # [O,C,KH,KW]
        lwf = lwf.transpose(1, 2, 3, 0).reshape(C, KH * KW, O)    # [C,9,O]
        w_terms = _bf16_terms(lwf)
        xt_terms = _bf16_terms(fx_r)
        both_split = len(w_terms) == 2 and len(xt_terms) == 2
        base = len(x_tables)
        x_tables.extend(xt_terms)
        for i_x in range(len(xt_terms)):
            for i_w, wt in enumerate(w_terms):
                if both_split and i_x == 1 and i_w == 1:
                    continue
                parts.append((base + i_x, wt))

    NP = len(parts)
    G = (NP + 1) // 2
    F = G * GSLEN + G * KH * KW * O + 2
    OFF_W = G * GSLEN
    OFF_B = OFF_W + G * KH * KW * O

    bias_u16 = bias.astype("<f4").view("<u2").reshape(O, 2)
    xmaps = np.stack([t[ixpad] for t in x_tables], axis=0)

    wreg = np.zeros((128, G * KH * KW * O), dtype=BF16)
    for p, (_, wt) in enumerate(parts):
        g, half = divmod(p, 2)
        rows = slice(half * C, half * C + C)
        for t in range(KH * KW):
            col = (g * KH * KW + t) * O
            wreg[rows, col:col + O] = wt[:, t, :]

    in_maps = []
    for c in range(N_CORES):
        b, half_img = divmod(c, 2)
        h0 = half_img * HH
        xin = np.zeros((128, F), dtype=BF16)
        for p, (xi, _) in enumerate(parts):
            g, half = divmod(p, 2)
            rows = slice(half * C, half * C + C)
            xin[rows, g * GSLEN:(g + 1) * GSLEN] = \
                xmaps[xi, b, :, h0:h0 + HH + 2, :].reshape(C, GSLEN)
        xin[:, OFF_W:OFF_B] = wreg
        xin.view("<u2")[:O, OFF_B:OFF_B + 2] = bias_u16
        in_maps.append({"xin": xin})
    return G, in_maps


def _run_generic(x, weight, bias, lut, scale_x, qx, scale_w, qw):
    global _LAST_RESULT
    G, in_maps = _prepare_generic(x, weight, bias, lut, scale_x, qx, scale_w, qw)
    key = ("generic", G)
    if key not in _PROGRAM_CACHE:
        _PROGRAM_CACHE[key] = _build_generic_program(G)
    nc = _PROGRAM_CACHE[key]
    try:
        res = run_bass_kernel_spmd(nc, in_maps, core_ids=list(range(N_CORES)),
                                   trace=_TRACE)
    except ModuleNotFoundError:
        res = run_bass_kernel_spmd(nc, in_maps, core_ids=list(range(N_CORES)),
                                   trace=False)
    _LAST_RESULT = res

    out = np.empty((B, O, OH, OW), dtype=np.float32)
    for c in range(N_CORES):
        b, half_img = divmod(c, 2)
        h0 = half_img * HH
        out[b, :, h0:h0 + HH, :] = res.results[c]["out"].reshape(O, HH, OW)
    return out


def kernel(x: np.ndarray, weight: np.ndarray, bias: np.ndarray,
           lut: np.ndarray) -> np.ndarray:
    x = np.asarray(x, dtype=np.float32)
    weight = np.asarray(weight, dtype=np.float32)
    bias = np.asarray(bias, dtype=np.float32)
    lut = np.asarray(lut, dtype=np.float32)

    scale_x, qx, scale_w, qw = _quantize(x, weight)
    if _fast_path_ok(qx, lut):
        return _run_fast(x, weight, bias, lut, scale_x, qx, scale_w, qw)
    return _run_generic(x, weight, bias, lut, scale_x, qx, scale_w, qw)
